# revision 8
# baseline (speedup 1.0000x reference)
"""Adaptive average pooling (16,250,250,256) -> (16,7,7,256), NHWC, f32.

Sharding: data-parallel over batch — 2 images per NeuronCore, 8 cores,
no collectives; host concatenates the per-core outputs.

Per-core algorithm (memory-bound, so everything is built around clean,
contiguous DMA):
  - x tiles are loaded in the natural layout: H on partitions, (w,c) on
    the free dim -> each partition is ONE contiguous DRAM run (nw KB),
    which keeps HWDGE descriptor count at 128/DMA and engages all 16
    SDMA engines.
  - H-pooling on the TensorEngine: matmul with a [h,7] 0/1 bin-indicator
    weight matrix (fp32r: 1 cycle/row at N>=256) accumulating over the
    two h-partition-chunks into a PSUM slab [7, nw*256].
  - W-pooling: VectorE tensor_reduce over the w ranges of each col-bin
    straight from PSUM, GpSimd accumulates bins across w-chunks in SBUF.
  - Epilogue: one tensor_scalar per (batch, col-bin) applies
    1/(count_h[i]*count_w[j]); single contiguous output DMA per batch.
"""

import sys

for _p in ("/opt/trn_rl_repo",):
    if _p not in sys.path:
        sys.path.insert(0, _p)

import numpy as np

from concourse import bacc, mybir, tile
from concourse.bass_utils import run_bass_kernel_spmd

B, H, W, C = 16, 250, 250, 256
OUT_H = OUT_W = 7
NCORES = 8
BPC = B // NCORES  # batches per core

NW = 8  # w columns per chunk (8 KB per partition per tile)


def _bin_edges(in_size, out_size):
    scale = np.float32(in_size / out_size)
    idx = np.arange(out_size, dtype=np.float32)
    starts = (idx * scale).astype(np.int32)
    ends = np.ceil((idx + 1.0) * scale).astype(np.int32)
    return starts, ends


SX, EX = _bin_edges(H, OUT_H)
SY, EY = _bin_edges(W, OUT_W)
CH = EX - SX
CW = EY - SY

HCHUNKS = [(0, 128), (128, 122)]
WCHUNKS = [(i * NW, min(NW, W - i * NW)) for i in range((W + NW - 1) // NW)]

_NC_CACHE = []


def _build():
    nc = bacc.Bacc("TRN2", target_bir_lowering=False, debug=False,
                   num_devices=NCORES)
    f32 = mybir.dt.float32
    f32r = mybir.dt.float32r
    x = nc.dram_tensor("x", [BPC, H, W, C], f32r, kind="ExternalInput").ap()
    pt = nc.dram_tensor("pt", [2, 128, OUT_H], f32r,
                        kind="ExternalInput").ap()
    invch = nc.dram_tensor("invch", [OUT_H, 1], f32,
                           kind="ExternalInput").ap()
    out = nc.dram_tensor("out", [BPC, OUT_H, OUT_W, C], f32,
                         kind="ExternalOutput").ap()

    mult = mybir.AluOpType.mult
    add = mybir.AluOpType.add

    with tile.TileContext(nc) as tc:
        with tc.tile_pool(name="const", bufs=1) as cpool, \
             tc.tile_pool(name="xp", bufs=4) as xpool, \
             tc.tile_pool(name="rp", bufs=4) as rpool, \
             tc.tile_pool(name="ap", bufs=2) as apool, \
             tc.tile_pool(name="op", bufs=2) as opool, \
             tc.tile_pool(name="ps", bufs=2, space="PSUM") as pspool:
            ptts = []
            for hci, (h0, hp) in enumerate(HCHUNKS):
                ptt = cpool.tile([hp, OUT_H], f32r, name=f"pt{hci}")
                nc.sync.dma_start(ptt[:], pt[hci, 0:hp, :])
                ptts.append(ptt)
            invch_t = cpool.tile([OUT_H, 1], f32, name="invch_t")
            nc.sync.dma_start(invch_t[:], invch[:])

            for b in range(BPC):
                accs = []
                for j in range(OUT_W):
                    acc = apool.tile([OUT_H, C], f32, tag=f"acc{j}",
                                     name=f"acc{j}_{b}")
                    nc.gpsimd.memset(acc[:], 0.0)
                    accs.append(acc)

                for (w0, nw) in WCHUNKS:
                    xts = []
                    for hci, (h0, hp) in enumerate(HCHUNKS):
                        xt = xpool.tile([hp, nw * C], f32r, tag=f"x{hci}",
                                        name=f"x{hci}_{b}_{w0}")
                        src = x[b, h0:h0 + hp, w0:w0 + nw, :]
                        src = src.rearrange("h w c -> h (w c)")
                        eng = nc.sync if hci == 0 else nc.scalar
                        eng.dma_start(xt[:], src)
                        xts.append(xt)
                    slab = pspool.tile([OUT_H, nw * C], f32, tag="slab",
                                       name=f"slab_{b}_{w0}")
                    for n in range(nw * C // 512):
                        sl = slice(n * 512, (n + 1) * 512)
                        for hci in range(2):
                            nc.tensor.matmul(
                                slab[:, sl], ptts[hci][:], xts[hci][:, sl],
                                start=(hci == 0), stop=(hci == 1))
                    slab3 = slab.rearrange("p (w c) -> p c w", c=C)
                    for j in range(OUT_W):
                        lo = max(w0, int(SY[j]))
                        hi = min(w0 + nw, int(EY[j]))
                        if hi <= lo:
                            continue
                        red = rpool.tile([OUT_H, C], f32, tag="red",
                                         name=f"red_{b}_{w0}_{j}")
                        nc.vector.tensor_reduce(
                            red[:], slab3[:, :, lo - w0:hi - w0],
                            axis=mybir.AxisListType.X, op=add)
                        nc.gpsimd.tensor_add(accs[j][:], accs[j][:], red[:])

                osb = opool.tile([OUT_H, OUT_W * C], f32, tag="osb",
                                 name=f"osb{b}")
                for j in range(OUT_W):
                    nc.vector.tensor_scalar(
                        osb[:, j * C:(j + 1) * C], accs[j][:],
                        scalar1=invch_t[:], scalar2=1.0 / float(CW[j]),
                        op0=mult, op1=mult)
                nc.sync.dma_start(
                    out[b], osb.rearrange("i (j c) -> i j c", c=C))

    nc.compile()
    return nc


def _get_nc():
    if not _NC_CACHE:
        _NC_CACHE.append(_build())
    return _NC_CACHE[0]


def _consts_np():
    ptv = np.zeros((2, 128, OUT_H), dtype=np.float32)
    for hci, (h0, hp) in enumerate(HCHUNKS):
        for p in range(hp):
            h = h0 + p
            for i in range(OUT_H):
                if SX[i] <= h < EX[i]:
                    ptv[hci, p, i] = 1.0
    invchv = (1.0 / CH.astype(np.float32)).reshape(OUT_H, 1)
    return ptv, invchv


def run(x: np.ndarray, **spmd_kwargs):
    x = np.ascontiguousarray(x, dtype=np.float32)
    assert x.shape == (B, H, W, C), x.shape
    nc = _get_nc()
    ptv, invchv = _consts_np()
    in_maps = [{"x": x[i * BPC:(i + 1) * BPC], "pt": ptv, "invch": invchv}
               for i in range(NCORES)]
    res = run_bass_kernel_spmd(nc, in_maps, core_ids=list(range(NCORES)),
                               **spmd_kwargs)
    out = np.concatenate([res.results[i]["out"] for i in range(NCORES)],
                         axis=0)
    return out, res


def kernel(x: np.ndarray) -> np.ndarray:
    out, _ = run(x)
    return out
